# revision 129
# baseline (speedup 1.0000x reference)
"""Trainium2 Bass/Tile kernel: GroupNorm + MHA + proj + residual.

Distribution: pure data parallel, 16 batches / 8 cores = 2 per core.
155.6us (fp8-residual baseline) -> 143.1us CoreSim / 1.36e-2 rel err.

Design (the kernel is ACT+DVE elementwise-bound; cost of every vector op
is max-free-size columns x cycle_t + a fixed access fee, partitions are
free, and only ACT/DVE can read PSUM — Pool has no PSUM port):
  - qkv gen + PV run fp8e4 DoubleRow (0.5 cyc/col); scores run bf16
    (1 cyc/col): q/k evac to bf16 kills the old k-residual DVE pass and
    improves precision. Weights carry x4 into fp8's normal range (undone
    by the 0.25 evac scale); proj runs bf16.
  - softmax exp splits across ACT (true Exp, scale=1/A bias=-SHIFT, fp8
    out) and DVE (one-op Schraudolph: scores arrive in PSUM pre-scaled
    by A=8/ln2 via the q/k weights; (psum + Bc) max 0 -> uint8 bitcast
    as fp8e4 IS exp(s-SHIFT)). The ACT/DVE split is per-head
    error-diffused (WAS0/WAS1) against each head's fixed engine load.
  - vT carries 64 ones-columns: PV at M=128 emits the softmax
    denominators pre-broadcast into pv rows 64..127 at zero extra cost;
    normalization is one DVE reciprocal + one DVE multiply per head
    (DVE divide is not in the ISA; two PSUM inputs per op are illegal).
  - PSUM = 8 banks exactly: scores ring "sT" 3x[128,1024] (6 banks) +
    "pv" 1x[128,1024] (2). Ring depth 3 hides the scores-matmul+sem
    latency behind the ~1us exps. qkv/v/proj matmul+evac groups are
    TENANTS of the sT ring (single batched 1024-col evacuations); the
    tiny gn matmuls share the pv slot.
  - The Tile scheduler is greedy by READINESS (emission order only
    breaks ties), so placement is controlled by data: x(1) tiles are
    gated by a 1-element identity multiply depending on b0 ht state,
    keeping b1's DVE-only bn_stats out of the startup-critical stream;
    b1 stats are spread per-tile across heads (0,2)-(0,3).
  - Tail: per-half evac+DMA staggering on 3 DMA queues.
  - walrus constraints honored: Pool runs only big SBUF->SBUF ops (xn
    affines, memsets) + SWDGE DMAs; ACT stays on the exp_and_others
    table (Identity evacuations, rsqrt via DVE bit-trick + 1 Newton).
"""

import numpy as np
import ml_dtypes

import concourse.bass as bass
import concourse.mybir as mybir
import concourse.tile as tile
from concourse import bacc

F32 = mybir.dt.float32
F32R = mybir.dt.float32r
FP8 = mybir.dt.float8e4
U8 = mybir.dt.uint8
AF = mybir.ActivationFunctionType
OP = mybir.AluOpType
PM = mybir.MatmulPerfMode
E4 = ml_dtypes.float8_e4m3

B, C, HH, WW = 16, 512, 32, 32
T = HH * WW            # 1024
NH, HD = 8, 64
N_CORES = 8
BPC = B // N_CORES     # 2
CT = C // 128          # 4 channel tiles
GROUPS = 32
GS = C // GROUPS       # 16
GPT = 128 // GS        # 8
EPS = 1e-5
SCALE = float(HD) ** -0.25

# schraudolph constants (see notes): psum holds A*score (A folded in weights)
A_EXP = 8.0 / np.log(2.0)          # 11.54156
SQRT_A = float(np.sqrt(A_EXP))     # 3.397287
SHIFT = 3.6
# hardware converts f32->u8 with round-to-nearest (CoreSim truncates);
# bias tuned for the hardware semantics: 56.0 + sigma(gm=1) - no +0.5
B_U8 = 56.0 - 0.3
BC_ADD = B_U8 - A_EXP * SHIFT      # 14.650383

# exp engine pattern per (h, sc): 'A' ACT, 'D' DVE, 'P' Pool.
# Ratios tuned against measured fixed per-engine load (error-diffusion).
def _mk_pat(wa, wd, ws):
    acc = {"A": 0.0, "D": 0.0, "S": 0.0}
    w = {"A": wa, "D": wd, "S": ws}
    out = []
    for _ in range(64):
        for k in w:
            acc[k] += w[k]
        pick = max(acc, key=lambda k: acc[k])
        acc[pick] -= 1.0
        out.append(pick)
    return ["".join(out[h * 8:(h + 1) * 8]) for h in range(8)]


# per-head ACT share of the 8 exp ops, derived from each head's fixed
# engine load (qkv/proj-evac fills on ACT/DVE, recip+mult on DVE, b1
# GroupNorm stats on DVE during heads (0,2)-(0,4)):
#   a = (8*1192 + fixedD - fixedA) / (1038 + 1192)
WAS0 = [0.56, 0.55, 0.76, 0.76, 0.74, 0.61, 0.55, 0.55]
WAS1 = [0.56, 0.68, 0.68, 0.68, 0.68, 0.67, 0.67, 0.67]


def _mk_pat_per_head(was0, was1):
    accA = 0.0
    out = []
    for wa in was0 + was1:
        for _ in range(8):
            accA += wa
            if accA >= 0.5:
                out.append("A")
                accA -= 1.0
            else:
                out.append("D")
    return [["".join(out[b * 64 + h * 8:b * 64 + (h + 1) * 8])
             for h in range(8)] for b in range(2)]


PAT = _mk_pat_per_head(WAS0, WAS1)
LOOKN = 5


def _build_body(ctx, tc, d):
    nc = tc.nc
    sb = ctx.enter_context(tc.tile_pool(name="sb", bufs=1))
    const = ctx.enter_context(tc.tile_pool(name="const", bufs=1))
    ps = ctx.enter_context(tc.tile_pool(name="ps", space="PSUM", bufs=1))

    S = [dict() for _ in range(BPC)]

    # ---- input DMAs ----------------------------------------------------
    # SP queue: x(0) chunks, qkv weights, x(1) chunks (in need order).
    # Pool queue (idle early): small consts + proj weights.
    S[1]["x"] = []
    for k in range(CT):
        xk = sb.tile([128, T], F32, name=f"x1_{k}", tag=f"x{k}", bufs=2)
        S[1]["x"].append(xk)
    gmask = const.tile([128, GPT], F32, name="gmask")
    nc.gpsimd.dma_start(out=gmask, in_=d["gmask"])
    bmask = const.tile([GPT, 128], F32, name="bmask")
    nc.gpsimd.dma_start(out=bmask, in_=d["bmask"])
    nwc = const.tile([128, CT], F32, name="nwc")
    nc.gpsimd.dma_start(out=nwc, in_=d["nw_cols"])
    nbc = const.tile([128, CT], F32, name="nbc")
    nc.gpsimd.dma_start(out=nbc, in_=d["nb_cols"])
    S[0]["x"] = []
    for k in range(CT):
        xk = sb.tile([128, T], F32, name=f"x0_{k}", tag=f"x{k}", bufs=2)
        # tile 0 in quarters: its first bn_stats gates the whole startup
        # chain, so land the first piece as early as possible
        parts = 4 if k == 0 else 2
        for p in range(parts):
            w = T // parts
            eng = nc.sync if k < 2 else nc.scalar
            eng.dma_start(
                out=xk[:, p * w:(p + 1) * w],
                in_=d["x"][0, k * 128:(k + 1) * 128, p * w:(p + 1) * w],
            )
        S[0]["x"].append(xk)
    qkv_w8 = []
    for j in range(2):
        w = const.tile([128, 2, 3 * C], FP8, name=f"qkv_w8_{j}")
        (nc.sync if j == 0 else nc.scalar).dma_start(out=w, in_=d["qkv_w8"][j])
        qkv_w8.append(w)
    qkb = const.tile([128, 2 * CT], F32, name="qkb")
    nc.sync.dma_start(out=qkb, in_=d["qk_bias_cols"])
    pbc = const.tile([128, CT], F32, name="pbc")
    proj_wbf = [
        const.tile([128, C], mybir.dt.bfloat16, name=f"proj_wbf_{kk}")
        for kk in range(CT)
    ]
    ident = const.tile([128, 128], F32, name="ident")

    def emit_proj_dmas():
        nc.gpsimd.dma_start(out=pbc, in_=d["pb_cols"])
        for kk in range(CT):
            nc.gpsimd.dma_start(out=proj_wbf[kk], in_=d["proj_wbf"][kk])
        nc.gpsimd.dma_start(out=ident, in_=d["ident"])

    def emit_x1_dmas():
        for k in range(0, CT):
            for half in range(2):
                nc.gpsimd.dma_start(
                    out=S[1]["x"][k][:, half * 512:(half + 1) * 512],
                    in_=d["x"][1, k * 128:(k + 1) * 128,
                               half * 512:(half + 1) * 512],
                )

    def emit_x1_gate(ks, hti, row=0):
        """The scheduler is greedy by readiness and DMAs have no deps, so
        x(1) lands early and b1's bn_stats would jump the startup-critical
        DVE stream. Gate: a 1-element identity multiply on each x1 tile
        whose other operand depends on b0 ht state (row picks which head's
        norm gates it) — b1 stats can't start until that norm is done."""
        htb = S[0]["ht"][hti]
        gate = sb.tile([1, 1], F32, name=f"gate{ks[0]}", tag="gate", bufs=2)
        nc.vector.tensor_scalar(out=gate, in0=htb[row:row + 1, 0:1],
                                scalar1=0.0, scalar2=1.0,
                                op0=OP.mult, op1=OP.add)
        for k in ks:
            for half in range(2):
                xs = S[1]["x"][k][0:1, half * 512:half * 512 + 1]
                nc.vector.tensor_tensor(out=xs, in0=xs, in1=gate, op=OP.mult)

    bcol = const.tile([128, 1], F32, name="bcol")
    nc.vector.memset(bcol, BC_ADD)
    negsh = const.tile([128, 1], F32, name="negsh")
    nc.vector.memset(negsh, -SHIFT)
    zcol = const.tile([128, 1], F32, name="zcol")
    nc.vector.memset(zcol, 0.0)

    # ---- emitters ------------------------------------------------------
    def emit_gn_stats(b, k):
        """per-x-tile GroupNorm stats: 2 bn_stats + aggr + s2 + group
        matmul into the shared ge accumulator (split out so b=1's stats
        can be spread across b=0's heads)."""
        S[b][f"done_gnst{k}"] = True
        x = S[b]["x"]
        ge = S[b].get("ge")
        if ge is None:
            ge = sb.tile([GPT, CT, 2], F32, name=f"ge{b}", tag="ge", bufs=2)
            S[b]["ge"] = ge
        parts = 4 if (b == 0 and k == 0) else 2
        st = sb.tile([128, parts, 6], F32, name=f"st{b}_{k}", tag="st",
                     bufs=2)
        for p in range(parts):
            w = T // parts
            nc.vector.bn_stats(out=st[:, p, :], in_=x[k][:, p * w:(p + 1) * w])
        mv = sb.tile([128, 2], F32, name=f"mv{b}_{k}", tag="mv", bufs=2)
        nc.vector.bn_aggr(out=mv, in_=st)
        # in-place: mv[:,1] = mean^2 + var, so mv feeds the group matmul
        # directly as [mean, E[x^2]] (skips a per-tile copy)
        nc.vector.scalar_tensor_tensor(
            out=mv[:, 1:2], in0=mv[:, 0:1], scalar=mv[:, 0:1],
            in1=mv[:, 1:2], op0=OP.mult, op1=OP.add,
        )
        gp = ps.tile([GPT, 2], F32, name=f"gp{b}_{k}", tag="pv", bufs=1)
        nc.tensor.matmul(gp, gmask, mv, start=True, stop=True)
        nc.vector.tensor_copy(out=ge[:, k, :], in_=gp)

    def emit_gn(b):
        """GroupNorm stats -> xn fp8 pair bufs.

        Small glue stays on DVE: walrus codegen rejects sub-column Pool
        tensor ops (and bn_stats/bn_aggr are DVE-only); only the big xn
        affine passes go to Pool for b=1."""
        ve = nc.vector
        x = S[b]["x"]
        ge = S[b].get("ge")
        if ge is None:
            ge = sb.tile([GPT, CT, 2], F32, name=f"ge{b}", tag="ge", bufs=2)
            S[b]["ge"] = ge
        for k in range(CT):
            if S[b].get(f"done_gnst{k}"):
                continue
            emit_gn_stats(b, k)
        gst = sb.tile([GPT, CT, 2], F32, name=f"gst{b}", tag="gst", bufs=2)
        gvar = sb.tile([GPT, CT], F32, name=f"gvar{b}", tag="gvar", bufs=2)
        ve.tensor_mul(gvar, ge[:, :, 0], ge[:, :, 0])
        ve.tensor_sub(gvar, ge[:, :, 1], gvar)
        ve.tensor_scalar_add(gvar, gvar, EPS)
        if True:
            # bit-trick rsqrt + TWO Newton steps (rel err ~3e-6): removes
            # every ACT Sqrt so the Exp table is loaded once and never
            # displaced
            I32 = mybir.dt.int32
            y0 = sb.tile([GPT, CT], F32, name=f"y0_{b}", tag="y0", bufs=1)
            nc.vector.tensor_scalar(
                out=y0.bitcast(I32), in0=gvar.bitcast(I32),
                scalar1=1, scalar2=None, op0=OP.logical_shift_right)
            nc.vector.tensor_scalar(
                out=y0.bitcast(I32), in0=y0.bitcast(I32),
                scalar1=-1, scalar2=0x5F3759DF, op0=OP.mult, op1=OP.add)
            t2 = sb.tile([GPT, CT], F32, name=f"t2_{b}", tag="t2", bufs=1)
            for _ in range(1):
                ve.tensor_mul(t2, y0, y0)
                ve.tensor_mul(t2, t2, gvar)
                ve.tensor_scalar(
                    out=t2, in0=t2, scalar1=-0.5, scalar2=1.5,
                    op0=OP.mult, op1=OP.add)
                ve.tensor_mul(y0, y0, t2)
            ve.tensor_copy(out=gst[:, :, 1], in_=y0)
        ve.tensor_copy(out=gst[:, :, 0], in_=ge[:, :, 0])

        xn = [
            sb.tile([128, 2, T], FP8, name=f"xn{b}_{j}", tag=f"xn{j}", bufs=2)
            for j in range(2)
        ]
        for k in range(CT):
            cps = ps.tile([128, 2], F32, name=f"cps{b}_{k}", tag="pv", bufs=1)
            nc.tensor.matmul(cps, bmask, gst[:, k, :], start=True, stop=True)
            Ak = sb.tile([128, 1], F32, name=f"A{b}_{k}", tag=f"A{k}", bufs=2)
            Bk = sb.tile([128, 1], F32, name=f"B{b}_{k}", tag=f"B{k}", bufs=2)
            nc.vector.tensor_mul(Ak, cps[:, 1:2], nwc[:, k:k + 1])
            nc.vector.tensor_mul(Bk, cps[:, 0:1], Ak)
            nc.vector.tensor_sub(Bk, nbc[:, k:k + 1], Bk)
            # b0 startup: alternate ACT/DVE so xn wall time is ~2 ops;
            # b1: all Pool (ACT/DVE are saturated mid-stream)
            if b == 0 and k % 2 == 0:
                nc.scalar.activation(
                    out=xn[k // 2][:, k % 2, :], in_=x[k],
                    func=AF.Identity, scale=Ak, bias=Bk,
                )
            else:
                xeng = nc.vector if b == 0 else nc.gpsimd
                xeng.tensor_scalar(
                    out=xn[k // 2][:, k % 2, :], in0=x[k],
                    scalar1=Ak, scalar2=Bk, op0=OP.mult, op1=OP.add,
                )
        S[b]["xn"] = xn

    def emit_qk(b, i, split=False, eng=None):
        """q tile (i<4) or k tile (i>=4), evacuated to bf16 for the scores
        matmul (weights carry x4 into fp8's normal range; evac scale 0.25).
        The PSUM tile is a tenant of the sT ring; both n-halves land in one
        slot so the evacuation is a single 1024-col op (split=True keeps
        per-half evacs so the first scores can start off the n0 half).
        eng=nc.vector routes the evac to DVE (startup balancing)."""
        xn = S[b]["xn"]
        is_k = i >= 4
        tag = f"k{i-4}" if is_k else f"q{i}"
        nm = f"k{b}_{i-4}" if is_k else f"q{b}_{i}"
        dst = sb.tile([128, T], mybir.dt.bfloat16, name=nm, tag=tag, bufs=2)
        S[b].setdefault("k" if is_k else "q", {})[i - 4 if is_k else i] = dst

        def evac(o, m):
            if eng is nc.vector:
                nc.vector.tensor_scalar(
                    out=o, in0=m, scalar1=0.25, scalar2=qkb[:, i:i + 1],
                    op0=OP.mult, op1=OP.add)
            else:
                nc.scalar.activation(out=o, in_=m, func=AF.Identity,
                                     bias=qkb[:, i:i + 1], scale=0.25)

        mm = ps.tile([128, T], F32, name=f"qk{b}_{i}", tag="sT", bufs=3)
        for n in range(2):
            for j in range(2):
                nc.tensor.matmul(
                    mm[:, n * 512:(n + 1) * 512],
                    qkv_w8[j][:, :, i * 128:(i + 1) * 128],
                    xn[j][:, :, n * 512:(n + 1) * 512],
                    start=(j == 0), stop=(j == 1), perf_mode=PM.DoubleRow,
                )
            if split:
                evac(dst[:, n * 512:(n + 1) * 512],
                     mm[:, n * 512:(n + 1) * 512])
        if not split:
            evac(dst, mm)

    def emit_vT_init(b):
        S[b]["vT"] = sb.tile([128, 8, NH, 2 * HD], FP8, name=f"vT{b}",
                             tag="vT", bufs=2)
        # columns HD..2HD are ones: PV at M=128 emits the softmax
        # denominators pre-broadcast into pv rows 64..128 for free
        nc.gpsimd.memset(S[b]["vT"][:, :, :, HD:2 * HD], 1.0)

    def emit_v(b, mp, eng=None):
        """vT s-chunk pair (2*mp, 2*mp+1) via one sT-ring slot."""
        xn = S[b]["xn"]
        if "vT" not in S[b]:
            emit_vT_init(b)
        vT = S[b]["vT"]
        mm = ps.tile([128, T], F32, name=f"v{b}_{mp}", tag="sT", bufs=3)
        for two in range(2):
            mt = 2 * mp + two
            for j in range(2):
                nc.tensor.matmul(
                    mm[:, two * 512:(two + 1) * 512],
                    xn[j][:, :, mt * 128:(mt + 1) * 128],
                    qkv_w8[j][:, :, 2 * C:3 * C],
                    start=(j == 0), stop=(j == 1), perf_mode=PM.DoubleRow,
                )
        vout = vT[:, 2 * mp:2 * mp + 2, :, 0:HD]
        vin = mm.rearrange("p (two h e) -> p two h e", two=2, h=NH)
        if eng is nc.vector:
            nc.vector.tensor_scalar(out=vout, in0=vin, scalar1=0.25,
                                    scalar2=0.0, op0=OP.mult, op1=OP.add)
        else:
            nc.scalar.activation(out=vout, in_=vin,
                                 func=AF.Identity, bias=zcol, scale=0.25)


    def emit_scores(b, h, sc):
        if S[b].get(f"done_s{h}_{sc}"):
            return
        S[b][f"done_s{h}_{sc}"] = True
        qt = S[b]["q"][h // 2]
        kt = S[b]["k"][h // 2]
        r0 = (h % 2) * 64
        kv = kt.rearrange("p (c m) -> p c m", m=128)  # [128, 8, 128]
        w = kv[r0:r0 + 64, sc, :]  # [64, 128] bf16
        sT = ps.tile([128, T], F32, name=f"sT{b}_{h}_{sc}", tag="sT", bufs=3)
        for n in range(2):
            qs = qt[r0:r0 + 64, n * 512:(n + 1) * 512]
            nc.tensor.matmul(sT[:, n * 512:(n + 1) * 512], w, qs,
                             start=True, stop=True)
        S[b][f"sT{h}_{sc}"] = sT

    def emit_expish(b, h, sc):
        if S[b].get(f"done_e{h}_{sc}"):
            return
        S[b][f"done_e{h}_{sc}"] = True
        pair, slot = sc // 2, sc % 2
        if slot == 0:
            S[b][f"pT{h}_{pair}"] = sb.tile(
                [128, 2, T], U8, name=f"pT{b}_{h}_{pair}", tag="pT", bufs=4)
        pt = S[b][f"pT{h}_{pair}"]
        sT = S[b].pop(f"sT{h}_{sc}")
        eng = PAT[b][h][sc]
        if eng == "A":
            nc.scalar.activation(out=pt[:, slot, :].bitcast(FP8), in_=sT,
                                 func=AF.Exp, bias=negsh, scale=1.0 / A_EXP)
        elif eng == "S":
            # split tile: ACT takes half 0, DVE half 1 (frees the sT slot
            # in ~half the time and spreads the work)
            nc.scalar.activation(out=pt[:, slot, 0:512].bitcast(FP8),
                                 in_=sT[:, 0:512],
                                 func=AF.Exp, bias=negsh, scale=1.0 / A_EXP)
            nc.vector.tensor_scalar(out=pt[:, slot, 512:1024],
                                    in0=sT[:, 512:1024],
                                    scalar1=bcol, scalar2=0.0,
                                    op0=OP.add, op1=OP.max)
        else:
            nc.vector.tensor_scalar(out=pt[:, slot, :], in0=sT,
                                    scalar1=bcol, scalar2=0.0,
                                    op0=OP.add, op1=OP.max)

    def emit_pv(b, h, pair):
        if S[b].get(f"done_p{h}_{pair}"):
            return
        S[b][f"done_p{h}_{pair}"] = True
        if pair == 0:
            S[b][f"pv{h}"] = ps.tile([128, T], F32, name=f"pv{b}_{h}",
                                     tag="pv", bufs=1)
        pv = S[b][f"pv{h}"]
        pt = S[b].pop(f"pT{h}_{pair}")
        w = S[b]["vT"][:, 2 * pair:2 * pair + 2, h, :]  # [128, 2, 65]
        for n in range(2):
            nc.tensor.matmul(
                pv[:, n * 512:(n + 1) * 512], w,
                pt.bitcast(FP8)[:, :, n * 512:(n + 1) * 512],
                start=(pair == 0), stop=False, skip_group_check=True,
                perf_mode=PM.DoubleRow,
            )

    def emit_stg(b, h, split=False):
        """reciprocal of the ones-trick denominators (DVE: divide is not in
        the DVE ISA, reciprocal+multiply is the only PSUM-legal route).
        split=True (last head): per-n-half so the tail proj pipeline can
        start off the n0 half ~1.2us earlier."""
        pv = S[b][f"pv{h}"]
        rstg = sb.tile([HD, T], F32, name=f"rstg{b}_{h}", tag="rstg", bufs=2)
        if split:
            for n in range(2):
                ns = slice(n * 512, (n + 1) * 512)
                nc.vector.reciprocal(out=rstg[:, ns], in_=pv[HD:2 * HD, ns])
        else:
            nc.vector.reciprocal(out=rstg, in_=pv[HD:2 * HD, :])
        S[b][f"stg{h}"] = rstg

    def emit_norm(b, h, split=False):
        if "ht" not in S[b]:
            S[b]["ht"] = [
                sb.tile([128, T], mybir.dt.bfloat16, name=f"ht{b}_{j}",
                        tag=f"ht{j}", bufs=2)
                for j in range(CT)
            ]
        pv = S[b].pop(f"pv{h}")
        rstg = S[b].pop(f"stg{h}")
        htb = S[b]["ht"][h // 2]
        r0 = (h % 2) * 64
        if split:
            for n in range(2):
                ns = slice(n * 512, (n + 1) * 512)
                nc.vector.tensor_tensor(
                    out=htb[r0:r0 + 64, ns], in0=pv[0:HD, ns],
                    in1=rstg[:, ns], op=OP.mult,
                )
        else:
            nc.vector.tensor_tensor(
                out=htb[r0:r0 + 64, :], in0=pv[0:HD, :], in1=rstg,
                op=OP.mult,
            )

    def emit_head(b, h, look=None, pre=None, fills=None, norm_split=False):
        """pv emission runs one pair behind exp so the in-order PE queue has
        score work in front of each (potentially blocking) pv matmul.
        fills: {sc: closure} emitted right after that chunk's exp (slot-ring
        friendly positions for qkv-gen tenancies)."""
        fills = fills or {}
        if pre:
            for f in pre:
                f()

        def chunk(sc):
            emit_scores(b, h, sc)
            emit_expish(b, h, sc)
            if sc in fills:
                fills[sc]()

        for sc in range(4):
            chunk(sc)
        emit_pv(b, h, 0)
        for sc in (4, 5):
            chunk(sc)
        emit_pv(b, h, 1)
        for sc in (6, 7):
            chunk(sc)
        emit_pv(b, h, 2)
        if look:
            look()
        emit_pv(b, h, 3)
        emit_stg(b, h, split=norm_split)
        emit_norm(b, h, split=norm_split)

    def look_scores(b, h, n=LOOKN):
        def f():
            for sc in range(n):
                emit_scores(b, h, sc)
                emit_expish(b, h, sc)
        return f

    def emit_proj_head(b, m, kks, tag="sT", xpe=False):
        """first kk contractions of proj m-tile into a persistent PSUM
        tenancy (kk=3 + evac follow in emit_proj once the last norm is
        emitted). tag="pv" reuses the pv slot freed by the final norm.
        xpe=True opens the group with an identity f32 matmul of the x
        residual so the final evac needs no tensor-tensor add."""
        ht = S[b]["ht"]
        x = S[b]["x"]
        bufs = 3 if tag == "sT" else 1
        pj = ps.tile([128, T], F32, name=f"pj{b}_{m}", tag=tag, bufs=bufs)
        S[b][f"pj{m}"] = pj
        S[b][f"pjkk{m}"] = len(kks)
        for n in range(2):
            ns = slice(n * 512, (n + 1) * 512)
            if xpe:
                # f32r: 1 cyc/col at >=256 cols (213ns vs 853 f32), ~2^-19
                # rounding — plenty for the residual passthrough
                nc.tensor.matmul(pj[:, ns], ident.bitcast(F32R),
                                 x[m][:, ns].bitcast(F32R),
                                 start=True, stop=False,
                                 skip_group_check=True)
            for kk in kks:
                nc.tensor.matmul(
                    pj[:, ns],
                    proj_wbf[kk][:, m * 128:(m + 1) * 128],
                    ht[kk][:, ns],
                    start=(kk == 0 and not xpe), stop=False,
                    skip_group_check=True,
                )

    def emit_proj(b, m, split=False, dq=None, xpe=False):
        """proj m-tile via one sT-ring tenancy. split=True staggers evac+DMA
        per n-half (tail pipelining); dq picks the DMA queue engine.
        xpe=True pre-adds the x residual into the PSUM group via an identity
        f32 matmul on PE (ready early, off the critical tail) so the evac is
        a plain ACT activation (+pbc) instead of a DVE-locked 3-input op."""
        ht = S[b]["ht"]
        x = S[b]["x"]
        y = sb.tile([128, T], F32, name=f"y{b}_{m}", tag="y", bufs=3)
        pj = S[b].pop(f"pj{m}", None)
        resumed = pj is not None
        kk0 = S[b].pop(f"pjkk{m}", CT - 1) if resumed else 0
        if pj is None:
            pj = ps.tile([128, T], F32, name=f"pj{b}_{m}", tag="sT", bufs=3)
        if dq is None:
            dq = nc.scalar if (b == 1 and m >= 2) else nc.sync
        for n in range(2):
            ns = slice(n * 512, (n + 1) * 512)
            if xpe and not resumed:
                nc.tensor.matmul(pj[:, ns], ident.bitcast(F32R),
                                 x[m][:, ns].bitcast(F32R),
                                 start=True, stop=False,
                                 skip_group_check=True)
            for kk in range(kk0, CT):
                nc.tensor.matmul(
                    pj[:, ns],
                    proj_wbf[kk][:, m * 128:(m + 1) * 128],
                    ht[kk][:, ns],
                    start=(kk == 0 and not xpe), stop=(kk == CT - 1),
                    skip_group_check=True,
                )
            if split:
                if xpe:
                    nc.scalar.activation(
                        out=y[:, ns], in_=pj[:, ns],
                        func=AF.Identity, bias=pbc[:, m:m + 1],
                    )
                else:
                    nc.vector.scalar_tensor_tensor(
                        out=y[:, ns], in0=pj[:, ns],
                        scalar=pbc[:, m:m + 1],
                        in1=x[m][:, ns], op0=OP.add, op1=OP.add,
                    )
                dq.dma_start(
                    out=d["out"][b, m * 128:(m + 1) * 128, ns],
                    in_=y[:, ns],
                )
        if not split:
            if xpe:
                nc.scalar.activation(
                    out=y, in_=pj, func=AF.Identity, bias=pbc[:, m:m + 1],
                )
            else:
                nc.vector.scalar_tensor_tensor(
                    out=y, in0=pj, scalar=pbc[:, m:m + 1],
                    in1=x[m], op0=OP.add, op1=OP.add,
                )
            dq.dma_start(
                out=d["out"][b, m * 128:(m + 1) * 128, :],
                in_=y,
            )

    # ---- schedule ------------------------------------------------------
    # head h of a batch needs q/k tile h//2 and (for pv) vT pairs; generate
    # just-in-time so the first scores start ASAP.
    emit_gn(0)
    emit_vT_init(0)
    emit_proj_dmas()
    emit_qk(0, 0, split=True)
    emit_qk(0, 4, split=True, eng=nc.vector)
    def look00():
        emit_v(0, 3)
        look_scores(0, 1)()

    emit_head(0, 0, look=look00,
              fills={1: lambda: emit_v(0, 0),
                     3: lambda: emit_v(0, 1, eng=nc.vector),
                     5: lambda: emit_v(0, 2)})
    emit_head(0, 1, look=look_scores(0, 2),
              pre=[emit_x1_dmas, lambda: emit_x1_gate([0, 1], 0)],
              fills={2: lambda: emit_qk(0, 1),
                     4: lambda: emit_gn_stats(1, 0),
                     5: lambda: emit_qk(0, 5),
                     6: lambda: emit_gn_stats(1, 1)})
    emit_head(0, 2, look=look_scores(0, 3),
              pre=[lambda: emit_x1_gate([2, 3], 0, row=64)],
              fills={2: lambda: emit_qk(0, 2),
                     4: lambda: emit_gn_stats(1, 2),
                     6: lambda: emit_gn_stats(1, 3)})
    emit_head(0, 3, look=look_scores(0, 4),
              fills={2: lambda: emit_qk(0, 6)})
    emit_head(0, 4, look=look_scores(0, 5),
              pre=[lambda: emit_gn(1)],
              fills={2: lambda: emit_qk(0, 3)})
    emit_head(0, 5, look=look_scores(0, 6),
              fills={2: lambda: emit_qk(0, 7)})
    emit_head(0, 6, look=look_scores(0, 7),
              fills={2: lambda: emit_qk(1, 0),
                     5: lambda: emit_qk(1, 4)})
    emit_head(0, 7, look=look_scores(1, 0, n=7),
              fills={2: lambda: emit_qk(1, 1),
                     5: lambda: emit_v(1, 0)})
    emit_head(1, 0, look=look_scores(1, 1, n=7),
              fills={1: lambda: emit_v(1, 1),
                     3: lambda: emit_v(1, 2, eng=nc.vector),
                     5: lambda: emit_qk(1, 5), 6: lambda: emit_v(1, 3)})
    emit_head(1, 1, look=look_scores(1, 2),
              fills={2: lambda: emit_proj(0, 0),
                     5: lambda: emit_qk(1, 2)})
    emit_head(1, 2, look=look_scores(1, 3, n=7),
              fills={2: lambda: emit_proj(0, 1),
                     5: lambda: emit_qk(1, 6)})
    emit_head(1, 3, look=look_scores(1, 4),
              fills={2: lambda: emit_proj(0, 2),
                     5: lambda: emit_qk(1, 3)})
    emit_head(1, 4, look=look_scores(1, 5),
              fills={2: lambda: emit_proj(0, 3),
                     5: lambda: emit_qk(1, 7)})
    emit_head(1, 5, look=look_scores(1, 6))
    emit_head(1, 6, look=look_scores(1, 7))
    emit_head(1, 7, norm_split=True)
    # kk 0..2 for three m-tiles first: they fill the PE window while the
    # in-order PE stream would otherwise block at m0's kk=3 (waits norm).
    # m3 reuses the pv slot freed by the final norm, with the x residual
    # pre-added on PE so its evac runs on the otherwise-idle ACT.
    tail_dq = [nc.sync, nc.scalar, nc.gpsimd, nc.sync]
    for m in range(CT):
        emit_proj(1, m, split=True, dq=tail_dq[m])


def build_nc():
    nc = bacc.Bacc("TRN2")
    d = {}
    d["x"] = nc.dram_tensor("x", [BPC, C, T], F32, kind="ExternalInput")[:]
    d["qkv_w8"] = nc.dram_tensor("qkv_w8", [2, 128, 2, 3 * C], FP8,
                                 kind="ExternalInput")[:]
    d["proj_wbf"] = nc.dram_tensor("proj_wbf", [CT, 128, C],
                                   mybir.dt.bfloat16,
                                   kind="ExternalInput")[:]
    d["qk_bias_cols"] = nc.dram_tensor("qk_bias_cols", [128, 2 * CT], F32,
                                       kind="ExternalInput")[:]
    d["nw_cols"] = nc.dram_tensor("nw_cols", [128, CT], F32,
                                  kind="ExternalInput")[:]
    d["nb_cols"] = nc.dram_tensor("nb_cols", [128, CT], F32,
                                  kind="ExternalInput")[:]
    d["pb_cols"] = nc.dram_tensor("pb_cols", [128, CT], F32,
                                  kind="ExternalInput")[:]
    d["gmask"] = nc.dram_tensor("gmask", [128, GPT], F32,
                                kind="ExternalInput")[:]
    d["bmask"] = nc.dram_tensor("bmask", [GPT, 128], F32,
                                kind="ExternalInput")[:]
    d["ident"] = nc.dram_tensor("ident", [128, 128], F32,
                                kind="ExternalInput")[:]
    d["out"] = nc.dram_tensor("out", [BPC, C, T], F32,
                              kind="ExternalOutput")[:]

    from contextlib import ExitStack

    with tile.TileContext(nc) as tc:
        with ExitStack() as ctx:
            _build_body(ctx, tc, d)
    nc.finalize()
    return nc


def host_inputs(x, norm_w, norm_b, qkv_w, qkv_b, proj_w, proj_b):
    f = np.float32
    perm = np.concatenate([
        np.concatenate([np.arange(3 * HD * h + j * HD, 3 * HD * h + (j + 1) * HD)
                        for h in range(NH)])
        for j in range(3)
    ])
    qkv_w = np.asarray(qkv_w, f)[perm].copy()
    qkv_b = np.asarray(qkv_b, f)[perm].copy()
    qk_fac = f(SCALE * SQRT_A)
    qkv_w[:2 * C] *= qk_fac
    qkv_b[:2 * C] *= qk_fac
    # fp8 range usage: all weight sections carry x4 (undone at evac) so
    # small weights stay out of fp8's denormal band
    qkv_w *= f(4.0)

    # weights in DoubleRow pair layout: [pair j][p, i, m] = wT[(2j+i)*128+p, m]
    qkv_wT = np.ascontiguousarray(qkv_w.T)            # [C, 3C]
    qkv_w8 = qkv_wT.reshape(2, 2, 128, 3 * C).transpose(0, 2, 1, 3)
    proj_wT = np.ascontiguousarray(np.asarray(proj_w, f).T)  # [C, C]

    import ml_dtypes as _mld
    consts = {
        "qkv_w8": np.ascontiguousarray(qkv_w8).astype(E4),
        "proj_wbf": np.ascontiguousarray(
            proj_wT.reshape(CT, 128, C)).astype(_mld.bfloat16),
        "qk_bias_cols": np.ascontiguousarray(
            qkv_b[:2 * C].reshape(2 * CT, 128).T),
        "nw_cols": np.ascontiguousarray(np.asarray(norm_w, f).reshape(CT, 128).T),
        "nb_cols": np.ascontiguousarray(np.asarray(norm_b, f).reshape(CT, 128).T),
        "pb_cols": np.ascontiguousarray(
            (np.asarray(proj_b, f)
             + np.asarray(proj_w, f) @ qkv_b[2 * C:]).reshape(CT, 128).T),
    }
    gmask = np.zeros((128, GPT), f)
    for p in range(128):
        gmask[p, p // GS] = 1.0 / GS
    consts["gmask"] = gmask
    consts["ident"] = np.eye(128, dtype=f)
    consts["bmask"] = np.ascontiguousarray((gmask.T > 0).astype(f))

    xs = np.ascontiguousarray(np.asarray(x, f).reshape(N_CORES, BPC, C, T))
    return xs, consts


_NC_CACHE = None


def kernel(x, norm_w, norm_b, qkv_w, qkv_b, proj_w, proj_b, num_heads=8, **_):
    from concourse.bass_utils import run_bass_kernel_spmd

    assert int(num_heads) == NH
    global _NC_CACHE
    if _NC_CACHE is None:
        _NC_CACHE = build_nc()
    nc = _NC_CACHE

    xs, consts = host_inputs(x, norm_w, norm_b, qkv_w, qkv_b, proj_w, proj_b)
    in_maps = [{"x": xs[i], **consts} for i in range(N_CORES)]
    res = run_bass_kernel_spmd(nc, in_maps, core_ids=list(range(N_CORES)))
    out = np.stack([res.results[i]["out"] for i in range(N_CORES)])
    return out.reshape(B, C, HH, WW)



# revision 130
# speedup vs baseline: 1.0044x; 1.0044x over previous
"""Trainium2 Bass/Tile kernel: GroupNorm + MHA + proj + residual.

Distribution: pure data parallel, 16 batches / 8 cores = 2 per core.
155.6us (fp8-residual baseline) -> 143.1us CoreSim / 1.36e-2 rel err.

Design (the kernel is ACT+DVE elementwise-bound; cost of every vector op
is max-free-size columns x cycle_t + a fixed access fee, partitions are
free, and only ACT/DVE can read PSUM — Pool has no PSUM port):
  - qkv gen + PV run fp8e4 DoubleRow (0.5 cyc/col); scores run bf16
    (1 cyc/col): q/k evac to bf16 kills the old k-residual DVE pass and
    improves precision. Weights carry x4 into fp8's normal range (undone
    by the 0.25 evac scale); proj runs bf16.
  - softmax exp splits across ACT (true Exp, scale=1/A bias=-SHIFT, fp8
    out) and DVE (one-op Schraudolph: scores arrive in PSUM pre-scaled
    by A=8/ln2 via the q/k weights; (psum + Bc) max 0 -> uint8 bitcast
    as fp8e4 IS exp(s-SHIFT)). The ACT/DVE split is per-head
    error-diffused (WAS0/WAS1) against each head's fixed engine load.
  - vT carries 64 ones-columns: PV at M=128 emits the softmax
    denominators pre-broadcast into pv rows 64..127 at zero extra cost;
    normalization is one DVE reciprocal + one DVE multiply per head
    (DVE divide is not in the ISA; two PSUM inputs per op are illegal).
  - PSUM = 8 banks exactly: scores ring "sT" 3x[128,1024] (6 banks) +
    "pv" 1x[128,1024] (2). Ring depth 3 hides the scores-matmul+sem
    latency behind the ~1us exps. qkv/v/proj matmul+evac groups are
    TENANTS of the sT ring (single batched 1024-col evacuations); the
    tiny gn matmuls share the pv slot.
  - The Tile scheduler is greedy by READINESS (emission order only
    breaks ties), so placement is controlled by data: x(1) tiles are
    gated by a 1-element identity multiply depending on b0 ht state,
    keeping b1's DVE-only bn_stats out of the startup-critical stream;
    b1 stats are spread per-tile across heads (0,2)-(0,3).
  - Tail: per-half evac+DMA staggering on 3 DMA queues.
  - walrus constraints honored: Pool runs only big SBUF->SBUF ops (xn
    affines, memsets) + SWDGE DMAs; ACT stays on the exp_and_others
    table (Identity evacuations, rsqrt via DVE bit-trick + 1 Newton).
"""

import numpy as np
import ml_dtypes

import concourse.bass as bass
import concourse.mybir as mybir
import concourse.tile as tile
from concourse import bacc

F32 = mybir.dt.float32
F32R = mybir.dt.float32r
FP8 = mybir.dt.float8e4
U8 = mybir.dt.uint8
AF = mybir.ActivationFunctionType
OP = mybir.AluOpType
PM = mybir.MatmulPerfMode
E4 = ml_dtypes.float8_e4m3

B, C, HH, WW = 16, 512, 32, 32
T = HH * WW            # 1024
NH, HD = 8, 64
N_CORES = 8
BPC = B // N_CORES     # 2
CT = C // 128          # 4 channel tiles
GROUPS = 32
GS = C // GROUPS       # 16
GPT = 128 // GS        # 8
EPS = 1e-5
SCALE = float(HD) ** -0.25

# schraudolph constants (see notes): psum holds A*score (A folded in weights)
A_EXP = 8.0 / np.log(2.0)          # 11.54156
SQRT_A = float(np.sqrt(A_EXP))     # 3.397287
SHIFT = 3.6
# hardware converts f32->u8 with round-to-nearest (CoreSim truncates);
# bias tuned for the hardware semantics: 56.0 + sigma(gm=1) - no +0.5
B_U8 = 56.0 - 0.3
BC_ADD = B_U8 - A_EXP * SHIFT      # 14.650383

# exp engine pattern per (h, sc): 'A' ACT, 'D' DVE, 'P' Pool.
# Ratios tuned against measured fixed per-engine load (error-diffusion).
def _mk_pat(wa, wd, ws):
    acc = {"A": 0.0, "D": 0.0, "S": 0.0}
    w = {"A": wa, "D": wd, "S": ws}
    out = []
    for _ in range(64):
        for k in w:
            acc[k] += w[k]
        pick = max(acc, key=lambda k: acc[k])
        acc[pick] -= 1.0
        out.append(pick)
    return ["".join(out[h * 8:(h + 1) * 8]) for h in range(8)]


# per-head ACT share of the 8 exp ops, derived from each head's fixed
# engine load (qkv/proj-evac fills on ACT/DVE, recip+mult on DVE, b1
# GroupNorm stats on DVE during heads (0,2)-(0,4)):
#   a = (8*1192 + fixedD - fixedA) / (1038 + 1192)
WAS0 = [0.56, 0.55, 0.76, 0.76, 0.74, 0.61, 0.55, 0.55]
WAS1 = [0.56, 0.68, 0.68, 0.68, 0.68, 0.67, 0.67, 0.67]


def _mk_pat_per_head(was0, was1):
    accA = 0.0
    out = []
    for wa in was0 + was1:
        for _ in range(8):
            accA += wa
            if accA >= 0.5:
                out.append("A")
                accA -= 1.0
            else:
                out.append("D")
    return [["".join(out[b * 64 + h * 8:b * 64 + (h + 1) * 8])
             for h in range(8)] for b in range(2)]


PAT = _mk_pat_per_head(WAS0, WAS1)
LOOKN = 5


def _build_body(ctx, tc, d):
    nc = tc.nc
    sb = ctx.enter_context(tc.tile_pool(name="sb", bufs=1))
    const = ctx.enter_context(tc.tile_pool(name="const", bufs=1))
    ps = ctx.enter_context(tc.tile_pool(name="ps", space="PSUM", bufs=1))

    S = [dict() for _ in range(BPC)]

    # ---- input DMAs ----------------------------------------------------
    # SP queue: x(0) chunks, qkv weights, x(1) chunks (in need order).
    # Pool queue (idle early): small consts + proj weights.
    S[1]["x"] = []
    for k in range(CT):
        xk = sb.tile([128, T], F32, name=f"x1_{k}", tag=f"x{k}", bufs=2)
        S[1]["x"].append(xk)
    gmask = const.tile([128, GPT], F32, name="gmask")
    nc.gpsimd.dma_start(out=gmask, in_=d["gmask"])
    bmask = const.tile([GPT, 128], F32, name="bmask")
    nc.gpsimd.dma_start(out=bmask, in_=d["bmask"])
    nwc = const.tile([128, CT], F32, name="nwc")
    nc.gpsimd.dma_start(out=nwc, in_=d["nw_cols"])
    nbc = const.tile([128, CT], F32, name="nbc")
    nc.gpsimd.dma_start(out=nbc, in_=d["nb_cols"])
    S[0]["x"] = []
    for k in range(CT):
        xk = sb.tile([128, T], F32, name=f"x0_{k}", tag=f"x{k}", bufs=2)
        # tile 0 in quarters: its first bn_stats gates the whole startup
        # chain, so land the first piece as early as possible
        parts = 4 if k == 0 else 2
        for p in range(parts):
            w = T // parts
            eng = nc.sync if k < 2 else nc.scalar
            eng.dma_start(
                out=xk[:, p * w:(p + 1) * w],
                in_=d["x"][0, k * 128:(k + 1) * 128, p * w:(p + 1) * w],
            )
        S[0]["x"].append(xk)
    qkv_w8 = []
    for j in range(2):
        w = const.tile([128, 2, 3 * C], FP8, name=f"qkv_w8_{j}")
        (nc.sync if j == 0 else nc.scalar).dma_start(out=w, in_=d["qkv_w8"][j])
        qkv_w8.append(w)
    qkb = const.tile([128, 2 * CT], F32, name="qkb")
    nc.sync.dma_start(out=qkb, in_=d["qk_bias_cols"])
    pbc = const.tile([128, CT], F32, name="pbc")
    proj_wbf = [
        const.tile([128, C], mybir.dt.bfloat16, name=f"proj_wbf_{kk}")
        for kk in range(CT)
    ]
    ident = const.tile([128, 128], F32, name="ident")

    def emit_proj_dmas():
        nc.gpsimd.dma_start(out=pbc, in_=d["pb_cols"])
        for kk in range(CT):
            nc.gpsimd.dma_start(out=proj_wbf[kk], in_=d["proj_wbf"][kk])
        nc.gpsimd.dma_start(out=ident, in_=d["ident"])

    def emit_x1_dmas():
        for k in range(0, CT):
            for half in range(2):
                nc.gpsimd.dma_start(
                    out=S[1]["x"][k][:, half * 512:(half + 1) * 512],
                    in_=d["x"][1, k * 128:(k + 1) * 128,
                               half * 512:(half + 1) * 512],
                )

    def emit_x1_gate(ks, hti, row=0):
        """The scheduler is greedy by readiness and DMAs have no deps, so
        x(1) lands early and b1's bn_stats would jump the startup-critical
        DVE stream. Gate: a 1-element identity multiply on each x1 tile
        whose other operand depends on b0 ht state (row picks which head's
        norm gates it) — b1 stats can't start until that norm is done."""
        htb = S[0]["ht"][hti]
        gate = sb.tile([1, 1], F32, name=f"gate{ks[0]}", tag="gate", bufs=2)
        nc.vector.tensor_scalar(out=gate, in0=htb[row:row + 1, 0:1],
                                scalar1=0.0, scalar2=1.0,
                                op0=OP.mult, op1=OP.add)
        for k in ks:
            for half in range(2):
                xs = S[1]["x"][k][0:1, half * 512:half * 512 + 1]
                nc.vector.tensor_tensor(out=xs, in0=xs, in1=gate, op=OP.mult)

    bcol = const.tile([128, 1], F32, name="bcol")
    nc.vector.memset(bcol, BC_ADD)
    negsh = const.tile([128, 1], F32, name="negsh")
    nc.vector.memset(negsh, -SHIFT)
    zcol = const.tile([128, 1], F32, name="zcol")
    nc.vector.memset(zcol, 0.0)

    # ---- emitters ------------------------------------------------------
    def emit_gn_stats(b, k):
        """per-x-tile GroupNorm stats: 2 bn_stats + aggr + s2 + group
        matmul into the shared ge accumulator (split out so b=1's stats
        can be spread across b=0's heads)."""
        S[b][f"done_gnst{k}"] = True
        x = S[b]["x"]
        ge = S[b].get("ge")
        if ge is None:
            ge = sb.tile([GPT, CT, 2], F32, name=f"ge{b}", tag="ge", bufs=2)
            S[b]["ge"] = ge
        parts = 4 if (b == 0 and k == 0) else 2
        st = sb.tile([128, parts, 6], F32, name=f"st{b}_{k}", tag="st",
                     bufs=2)
        for p in range(parts):
            w = T // parts
            nc.vector.bn_stats(out=st[:, p, :], in_=x[k][:, p * w:(p + 1) * w])
        mv = sb.tile([128, 2], F32, name=f"mv{b}_{k}", tag="mv", bufs=2)
        nc.vector.bn_aggr(out=mv, in_=st)
        # in-place: mv[:,1] = mean^2 + var, so mv feeds the group matmul
        # directly as [mean, E[x^2]] (skips a per-tile copy)
        nc.vector.scalar_tensor_tensor(
            out=mv[:, 1:2], in0=mv[:, 0:1], scalar=mv[:, 0:1],
            in1=mv[:, 1:2], op0=OP.mult, op1=OP.add,
        )
        gp = ps.tile([GPT, 2], F32, name=f"gp{b}_{k}", tag="pv", bufs=1)
        nc.tensor.matmul(gp, gmask, mv, start=True, stop=True)
        nc.vector.tensor_copy(out=ge[:, k, :], in_=gp)

    def emit_gn(b):
        """GroupNorm stats -> xn fp8 pair bufs.

        Small glue stays on DVE: walrus codegen rejects sub-column Pool
        tensor ops (and bn_stats/bn_aggr are DVE-only); only the big xn
        affine passes go to Pool for b=1."""
        ve = nc.vector
        x = S[b]["x"]
        ge = S[b].get("ge")
        if ge is None:
            ge = sb.tile([GPT, CT, 2], F32, name=f"ge{b}", tag="ge", bufs=2)
            S[b]["ge"] = ge
        for k in range(CT):
            if S[b].get(f"done_gnst{k}"):
                continue
            emit_gn_stats(b, k)
        gst = sb.tile([GPT, CT, 2], F32, name=f"gst{b}", tag="gst", bufs=2)
        gvar = sb.tile([GPT, CT], F32, name=f"gvar{b}", tag="gvar", bufs=2)
        ve.tensor_mul(gvar, ge[:, :, 0], ge[:, :, 0])
        ve.tensor_sub(gvar, ge[:, :, 1], gvar)
        ve.tensor_scalar_add(gvar, gvar, EPS)
        if True:
            # bit-trick rsqrt + TWO Newton steps (rel err ~3e-6): removes
            # every ACT Sqrt so the Exp table is loaded once and never
            # displaced
            I32 = mybir.dt.int32
            y0 = sb.tile([GPT, CT], F32, name=f"y0_{b}", tag="y0", bufs=1)
            nc.vector.tensor_scalar(
                out=y0.bitcast(I32), in0=gvar.bitcast(I32),
                scalar1=1, scalar2=None, op0=OP.logical_shift_right)
            nc.vector.tensor_scalar(
                out=y0.bitcast(I32), in0=y0.bitcast(I32),
                scalar1=-1, scalar2=0x5F3759DF, op0=OP.mult, op1=OP.add)
            t2 = sb.tile([GPT, CT], F32, name=f"t2_{b}", tag="t2", bufs=1)
            for _ in range(1):
                ve.tensor_mul(t2, y0, y0)
                ve.tensor_mul(t2, t2, gvar)
                ve.tensor_scalar(
                    out=t2, in0=t2, scalar1=-0.5, scalar2=1.5,
                    op0=OP.mult, op1=OP.add)
                ve.tensor_mul(y0, y0, t2)
            ve.tensor_copy(out=gst[:, :, 1], in_=y0)
        ve.tensor_copy(out=gst[:, :, 0], in_=ge[:, :, 0])

        xn = [
            sb.tile([128, 2, T], FP8, name=f"xn{b}_{j}", tag=f"xn{j}", bufs=2)
            for j in range(2)
        ]
        for k in range(CT):
            cps = ps.tile([128, 2], F32, name=f"cps{b}_{k}", tag="pv", bufs=1)
            nc.tensor.matmul(cps, bmask, gst[:, k, :], start=True, stop=True)
            Ak = sb.tile([128, 1], F32, name=f"A{b}_{k}", tag=f"A{k}", bufs=2)
            Bk = sb.tile([128, 1], F32, name=f"B{b}_{k}", tag=f"B{k}", bufs=2)
            nc.vector.tensor_mul(Ak, cps[:, 1:2], nwc[:, k:k + 1])
            nc.vector.tensor_mul(Bk, cps[:, 0:1], Ak)
            nc.vector.tensor_sub(Bk, nbc[:, k:k + 1], Bk)
            # b0 startup: alternate ACT/DVE so xn wall time is ~2 ops;
            # b1: all Pool (ACT/DVE are saturated mid-stream)
            if b == 0 and k % 2 == 0:
                nc.scalar.activation(
                    out=xn[k // 2][:, k % 2, :], in_=x[k],
                    func=AF.Identity, scale=Ak, bias=Bk,
                )
            else:
                xeng = nc.vector if b == 0 else nc.gpsimd
                xeng.tensor_scalar(
                    out=xn[k // 2][:, k % 2, :], in0=x[k],
                    scalar1=Ak, scalar2=Bk, op0=OP.mult, op1=OP.add,
                )
        S[b]["xn"] = xn

    def emit_qk(b, i, split=False, eng=None):
        """q tile (i<4) or k tile (i>=4), evacuated to bf16 for the scores
        matmul (weights carry x4 into fp8's normal range; evac scale 0.25).
        The PSUM tile is a tenant of the sT ring; both n-halves land in one
        slot so the evacuation is a single 1024-col op (split=True keeps
        per-half evacs so the first scores can start off the n0 half).
        eng=nc.vector routes the evac to DVE (startup balancing)."""
        xn = S[b]["xn"]
        is_k = i >= 4
        tag = f"k{i-4}" if is_k else f"q{i}"
        nm = f"k{b}_{i-4}" if is_k else f"q{b}_{i}"
        dst = sb.tile([128, T], mybir.dt.bfloat16, name=nm, tag=tag, bufs=2)
        S[b].setdefault("k" if is_k else "q", {})[i - 4 if is_k else i] = dst

        def evac(o, m):
            if eng is nc.vector:
                nc.vector.tensor_scalar(
                    out=o, in0=m, scalar1=0.25, scalar2=qkb[:, i:i + 1],
                    op0=OP.mult, op1=OP.add)
            else:
                nc.scalar.activation(out=o, in_=m, func=AF.Identity,
                                     bias=qkb[:, i:i + 1], scale=0.25)

        mm = ps.tile([128, T], F32, name=f"qk{b}_{i}", tag="sT", bufs=3)
        for n in range(2):
            for j in range(2):
                nc.tensor.matmul(
                    mm[:, n * 512:(n + 1) * 512],
                    qkv_w8[j][:, :, i * 128:(i + 1) * 128],
                    xn[j][:, :, n * 512:(n + 1) * 512],
                    start=(j == 0), stop=(j == 1), perf_mode=PM.DoubleRow,
                )
            if split:
                evac(dst[:, n * 512:(n + 1) * 512],
                     mm[:, n * 512:(n + 1) * 512])
        if not split:
            evac(dst, mm)

    def emit_vT_init(b):
        S[b]["vT"] = sb.tile([128, 8, NH, 2 * HD], FP8, name=f"vT{b}",
                             tag="vT", bufs=2)
        # columns HD..2HD are ones: PV at M=128 emits the softmax
        # denominators pre-broadcast into pv rows 64..128 for free
        nc.gpsimd.memset(S[b]["vT"][:, :, :, HD:2 * HD], 1.0)

    def emit_v(b, mp, eng=None):
        """vT s-chunk pair (2*mp, 2*mp+1) via one sT-ring slot."""
        xn = S[b]["xn"]
        if "vT" not in S[b]:
            emit_vT_init(b)
        vT = S[b]["vT"]
        mm = ps.tile([128, T], F32, name=f"v{b}_{mp}", tag="sT", bufs=3)
        for two in range(2):
            mt = 2 * mp + two
            for j in range(2):
                nc.tensor.matmul(
                    mm[:, two * 512:(two + 1) * 512],
                    xn[j][:, :, mt * 128:(mt + 1) * 128],
                    qkv_w8[j][:, :, 2 * C:3 * C],
                    start=(j == 0), stop=(j == 1), perf_mode=PM.DoubleRow,
                )
        vout = vT[:, 2 * mp:2 * mp + 2, :, 0:HD]
        vin = mm.rearrange("p (two h e) -> p two h e", two=2, h=NH)
        if eng is nc.vector:
            nc.vector.tensor_scalar(out=vout, in0=vin, scalar1=0.25,
                                    scalar2=0.0, op0=OP.mult, op1=OP.add)
        else:
            nc.scalar.activation(out=vout, in_=vin,
                                 func=AF.Identity, bias=zcol, scale=0.25)


    def emit_scores(b, h, sc):
        if S[b].get(f"done_s{h}_{sc}"):
            return
        S[b][f"done_s{h}_{sc}"] = True
        qt = S[b]["q"][h // 2]
        kt = S[b]["k"][h // 2]
        r0 = (h % 2) * 64
        kv = kt.rearrange("p (c m) -> p c m", m=128)  # [128, 8, 128]
        w = kv[r0:r0 + 64, sc, :]  # [64, 128] bf16
        sT = ps.tile([128, T], F32, name=f"sT{b}_{h}_{sc}", tag="sT", bufs=3)
        for n in range(2):
            qs = qt[r0:r0 + 64, n * 512:(n + 1) * 512]
            nc.tensor.matmul(sT[:, n * 512:(n + 1) * 512], w, qs,
                             start=True, stop=True)
        S[b][f"sT{h}_{sc}"] = sT

    def emit_expish(b, h, sc):
        if S[b].get(f"done_e{h}_{sc}"):
            return
        S[b][f"done_e{h}_{sc}"] = True
        pair, slot = sc // 2, sc % 2
        if slot == 0:
            S[b][f"pT{h}_{pair}"] = sb.tile(
                [128, 2, T], U8, name=f"pT{b}_{h}_{pair}", tag="pT", bufs=4)
        pt = S[b][f"pT{h}_{pair}"]
        sT = S[b].pop(f"sT{h}_{sc}")
        eng = PAT[b][h][sc]
        if eng == "A":
            nc.scalar.activation(out=pt[:, slot, :].bitcast(FP8), in_=sT,
                                 func=AF.Exp, bias=negsh, scale=1.0 / A_EXP)
        elif eng == "S":
            # split tile: ACT takes half 0, DVE half 1 (frees the sT slot
            # in ~half the time and spreads the work)
            nc.scalar.activation(out=pt[:, slot, 0:512].bitcast(FP8),
                                 in_=sT[:, 0:512],
                                 func=AF.Exp, bias=negsh, scale=1.0 / A_EXP)
            nc.vector.tensor_scalar(out=pt[:, slot, 512:1024],
                                    in0=sT[:, 512:1024],
                                    scalar1=bcol, scalar2=0.0,
                                    op0=OP.add, op1=OP.max)
        else:
            nc.vector.tensor_scalar(out=pt[:, slot, :], in0=sT,
                                    scalar1=bcol, scalar2=0.0,
                                    op0=OP.add, op1=OP.max)

    def emit_pv(b, h, pair):
        if S[b].get(f"done_p{h}_{pair}"):
            return
        S[b][f"done_p{h}_{pair}"] = True
        if pair == 0:
            S[b][f"pv{h}"] = ps.tile([128, T], F32, name=f"pv{b}_{h}",
                                     tag="pv", bufs=1)
        pv = S[b][f"pv{h}"]
        pt = S[b].pop(f"pT{h}_{pair}")
        w = S[b]["vT"][:, 2 * pair:2 * pair + 2, h, :]  # [128, 2, 65]
        for n in range(2):
            nc.tensor.matmul(
                pv[:, n * 512:(n + 1) * 512], w,
                pt.bitcast(FP8)[:, :, n * 512:(n + 1) * 512],
                start=(pair == 0), stop=False, skip_group_check=True,
                perf_mode=PM.DoubleRow,
            )

    def emit_stg(b, h, split=False):
        """reciprocal of the ones-trick denominators (DVE: divide is not in
        the DVE ISA, reciprocal+multiply is the only PSUM-legal route).
        split=True (last head): per-n-half so the tail proj pipeline can
        start off the n0 half ~1.2us earlier."""
        pv = S[b][f"pv{h}"]
        rstg = sb.tile([HD, T], F32, name=f"rstg{b}_{h}", tag="rstg", bufs=2)
        if split:
            for n in range(2):
                ns = slice(n * 512, (n + 1) * 512)
                nc.vector.reciprocal(out=rstg[:, ns], in_=pv[HD:2 * HD, ns])
        else:
            nc.vector.reciprocal(out=rstg, in_=pv[HD:2 * HD, :])
        S[b][f"stg{h}"] = rstg

    def emit_norm(b, h, split=False):
        if "ht" not in S[b]:
            S[b]["ht"] = [
                sb.tile([128, T], mybir.dt.bfloat16, name=f"ht{b}_{j}",
                        tag=f"ht{j}", bufs=2)
                for j in range(CT)
            ]
        pv = S[b].pop(f"pv{h}")
        rstg = S[b].pop(f"stg{h}")
        htb = S[b]["ht"][h // 2]
        r0 = (h % 2) * 64
        if split:
            for n in range(2):
                ns = slice(n * 512, (n + 1) * 512)
                nc.vector.tensor_tensor(
                    out=htb[r0:r0 + 64, ns], in0=pv[0:HD, ns],
                    in1=rstg[:, ns], op=OP.mult,
                )
        else:
            nc.vector.tensor_tensor(
                out=htb[r0:r0 + 64, :], in0=pv[0:HD, :], in1=rstg,
                op=OP.mult,
            )

    def emit_head(b, h, look=None, pre=None, fills=None, norm_split=False):
        """pv emission runs one pair behind exp so the in-order PE queue has
        score work in front of each (potentially blocking) pv matmul.
        fills: {sc: closure} emitted right after that chunk's exp (slot-ring
        friendly positions for qkv-gen tenancies)."""
        fills = fills or {}
        if pre:
            for f in pre:
                f()

        def chunk(sc):
            emit_scores(b, h, sc)
            emit_expish(b, h, sc)
            if sc in fills:
                fills[sc]()

        for sc in range(4):
            chunk(sc)
        emit_pv(b, h, 0)
        for sc in (4, 5):
            chunk(sc)
        emit_pv(b, h, 1)
        for sc in (6, 7):
            chunk(sc)
        emit_pv(b, h, 2)
        if look:
            look()
        emit_pv(b, h, 3)
        emit_stg(b, h, split=norm_split)
        emit_norm(b, h, split=norm_split)

    def look_scores(b, h, n=LOOKN):
        def f():
            for sc in range(n):
                emit_scores(b, h, sc)
                emit_expish(b, h, sc)
        return f

    def emit_proj_head(b, m, kks, tag="sT", xpe=False):
        """first kk contractions of proj m-tile into a persistent PSUM
        tenancy (kk=3 + evac follow in emit_proj once the last norm is
        emitted). tag="pv" reuses the pv slot freed by the final norm.
        xpe=True opens the group with an identity f32 matmul of the x
        residual so the final evac needs no tensor-tensor add."""
        ht = S[b]["ht"]
        x = S[b]["x"]
        bufs = 3 if tag == "sT" else 1
        pj = ps.tile([128, T], F32, name=f"pj{b}_{m}", tag=tag, bufs=bufs)
        S[b][f"pj{m}"] = pj
        S[b][f"pjkk{m}"] = len(kks)
        for n in range(2):
            ns = slice(n * 512, (n + 1) * 512)
            if xpe:
                # f32r: 1 cyc/col at >=256 cols (213ns vs 853 f32), ~2^-19
                # rounding — plenty for the residual passthrough
                nc.tensor.matmul(pj[:, ns], ident.bitcast(F32R),
                                 x[m][:, ns].bitcast(F32R),
                                 start=True, stop=False,
                                 skip_group_check=True)
            for kk in kks:
                nc.tensor.matmul(
                    pj[:, ns],
                    proj_wbf[kk][:, m * 128:(m + 1) * 128],
                    ht[kk][:, ns],
                    start=(kk == 0 and not xpe), stop=False,
                    skip_group_check=True,
                )

    def emit_proj(b, m, split=False, dq=None, xpe=False):
        """proj m-tile via one sT-ring tenancy. split=True staggers evac+DMA
        per n-half (tail pipelining); dq picks the DMA queue engine.
        xpe=True pre-adds the x residual into the PSUM group via an identity
        f32 matmul on PE (ready early, off the critical tail) so the evac is
        a plain ACT activation (+pbc) instead of a DVE-locked 3-input op."""
        ht = S[b]["ht"]
        x = S[b]["x"]
        y = sb.tile([128, T], F32, name=f"y{b}_{m}", tag="y", bufs=3)
        pj = S[b].pop(f"pj{m}", None)
        resumed = pj is not None
        kk0 = S[b].pop(f"pjkk{m}", CT - 1) if resumed else 0
        if pj is None:
            pj = ps.tile([128, T], F32, name=f"pj{b}_{m}", tag="sT", bufs=3)
        if dq is None:
            dq = nc.scalar if (b == 1 and m >= 2) else nc.sync
        for n in range(2):
            ns = slice(n * 512, (n + 1) * 512)
            if xpe and not resumed:
                nc.tensor.matmul(pj[:, ns], ident.bitcast(F32R),
                                 x[m][:, ns].bitcast(F32R),
                                 start=True, stop=False,
                                 skip_group_check=True)
            for kk in range(kk0, CT):
                nc.tensor.matmul(
                    pj[:, ns],
                    proj_wbf[kk][:, m * 128:(m + 1) * 128],
                    ht[kk][:, ns],
                    start=(kk == 0 and not xpe), stop=(kk == CT - 1),
                    skip_group_check=True,
                )
            if split:
                if xpe:
                    nc.scalar.activation(
                        out=y[:, ns], in_=pj[:, ns],
                        func=AF.Identity, bias=pbc[:, m:m + 1],
                    )
                else:
                    nc.vector.scalar_tensor_tensor(
                        out=y[:, ns], in0=pj[:, ns],
                        scalar=pbc[:, m:m + 1],
                        in1=x[m][:, ns], op0=OP.add, op1=OP.add,
                    )
                dq.dma_start(
                    out=d["out"][b, m * 128:(m + 1) * 128, ns],
                    in_=y[:, ns],
                )
        if not split:
            if xpe:
                nc.scalar.activation(
                    out=y, in_=pj, func=AF.Identity, bias=pbc[:, m:m + 1],
                )
            else:
                nc.vector.scalar_tensor_tensor(
                    out=y, in0=pj, scalar=pbc[:, m:m + 1],
                    in1=x[m], op0=OP.add, op1=OP.add,
                )
            dq.dma_start(
                out=d["out"][b, m * 128:(m + 1) * 128, :],
                in_=y,
            )

    # ---- schedule ------------------------------------------------------
    # head h of a batch needs q/k tile h//2 and (for pv) vT pairs; generate
    # just-in-time so the first scores start ASAP.
    emit_gn(0)
    emit_vT_init(0)
    emit_proj_dmas()
    emit_qk(0, 0, split=True)
    emit_qk(0, 4, split=True, eng=nc.vector)
    def look00():
        emit_v(0, 3)
        look_scores(0, 1)()

    emit_head(0, 0, look=look00,
              fills={1: lambda: emit_v(0, 0),
                     3: lambda: emit_v(0, 1, eng=nc.vector),
                     5: lambda: emit_v(0, 2)})
    emit_head(0, 1, look=look_scores(0, 2),
              pre=[emit_x1_dmas, lambda: emit_x1_gate([0, 1], 0)],
              fills={2: lambda: emit_qk(0, 1),
                     4: lambda: emit_gn_stats(1, 0),
                     5: lambda: emit_qk(0, 5),
                     6: lambda: emit_gn_stats(1, 1)})
    emit_head(0, 2, look=look_scores(0, 3),
              pre=[lambda: emit_x1_gate([2, 3], 0, row=64)],
              fills={2: lambda: emit_qk(0, 2),
                     4: lambda: emit_gn_stats(1, 2),
                     6: lambda: emit_gn_stats(1, 3)})
    emit_head(0, 3, look=look_scores(0, 4),
              fills={2: lambda: emit_qk(0, 6)})
    emit_head(0, 4, look=look_scores(0, 5),
              pre=[lambda: emit_gn(1)],
              fills={2: lambda: emit_qk(0, 3)})
    emit_head(0, 5, look=look_scores(0, 6),
              fills={2: lambda: emit_qk(0, 7)})
    emit_head(0, 6, look=look_scores(0, 7),
              fills={2: lambda: emit_qk(1, 0),
                     5: lambda: emit_qk(1, 4)})
    emit_head(0, 7, look=look_scores(1, 0, n=7),
              fills={2: lambda: emit_qk(1, 1),
                     5: lambda: emit_v(1, 0)})
    emit_head(1, 0, look=look_scores(1, 1),
              fills={1: lambda: emit_v(1, 1),
                     3: lambda: emit_v(1, 2, eng=nc.vector),
                     5: lambda: emit_qk(1, 5), 6: lambda: emit_v(1, 3)})
    emit_head(1, 1, look=look_scores(1, 2),
              fills={2: lambda: emit_proj(0, 0),
                     5: lambda: emit_qk(1, 2)})
    emit_head(1, 2, look=look_scores(1, 3),
              fills={2: lambda: emit_proj(0, 1),
                     5: lambda: emit_qk(1, 6)})
    emit_head(1, 3, look=look_scores(1, 4),
              fills={2: lambda: emit_proj(0, 2),
                     5: lambda: emit_qk(1, 3)})
    emit_head(1, 4, look=look_scores(1, 5),
              fills={2: lambda: emit_proj(0, 3),
                     5: lambda: emit_qk(1, 7)})
    emit_head(1, 5, look=look_scores(1, 6))
    emit_head(1, 6, look=look_scores(1, 7))
    emit_head(1, 7, norm_split=True)
    # kk 0..2 for three m-tiles first: they fill the PE window while the
    # in-order PE stream would otherwise block at m0's kk=3 (waits norm).
    # m3 reuses the pv slot freed by the final norm, with the x residual
    # pre-added on PE so its evac runs on the otherwise-idle ACT.
    tail_dq = [nc.sync, nc.scalar, nc.gpsimd, nc.sync]
    for m in range(CT):
        emit_proj(1, m, split=True, dq=tail_dq[m])


def build_nc():
    nc = bacc.Bacc("TRN2")
    d = {}
    d["x"] = nc.dram_tensor("x", [BPC, C, T], F32, kind="ExternalInput")[:]
    d["qkv_w8"] = nc.dram_tensor("qkv_w8", [2, 128, 2, 3 * C], FP8,
                                 kind="ExternalInput")[:]
    d["proj_wbf"] = nc.dram_tensor("proj_wbf", [CT, 128, C],
                                   mybir.dt.bfloat16,
                                   kind="ExternalInput")[:]
    d["qk_bias_cols"] = nc.dram_tensor("qk_bias_cols", [128, 2 * CT], F32,
                                       kind="ExternalInput")[:]
    d["nw_cols"] = nc.dram_tensor("nw_cols", [128, CT], F32,
                                  kind="ExternalInput")[:]
    d["nb_cols"] = nc.dram_tensor("nb_cols", [128, CT], F32,
                                  kind="ExternalInput")[:]
    d["pb_cols"] = nc.dram_tensor("pb_cols", [128, CT], F32,
                                  kind="ExternalInput")[:]
    d["gmask"] = nc.dram_tensor("gmask", [128, GPT], F32,
                                kind="ExternalInput")[:]
    d["bmask"] = nc.dram_tensor("bmask", [GPT, 128], F32,
                                kind="ExternalInput")[:]
    d["ident"] = nc.dram_tensor("ident", [128, 128], F32,
                                kind="ExternalInput")[:]
    d["out"] = nc.dram_tensor("out", [BPC, C, T], F32,
                              kind="ExternalOutput")[:]

    from contextlib import ExitStack

    with tile.TileContext(nc) as tc:
        with ExitStack() as ctx:
            _build_body(ctx, tc, d)
    nc.finalize()
    return nc


def host_inputs(x, norm_w, norm_b, qkv_w, qkv_b, proj_w, proj_b):
    f = np.float32
    perm = np.concatenate([
        np.concatenate([np.arange(3 * HD * h + j * HD, 3 * HD * h + (j + 1) * HD)
                        for h in range(NH)])
        for j in range(3)
    ])
    qkv_w = np.asarray(qkv_w, f)[perm].copy()
    qkv_b = np.asarray(qkv_b, f)[perm].copy()
    qk_fac = f(SCALE * SQRT_A)
    qkv_w[:2 * C] *= qk_fac
    qkv_b[:2 * C] *= qk_fac
    # fp8 range usage: all weight sections carry x4 (undone at evac) so
    # small weights stay out of fp8's denormal band
    qkv_w *= f(4.0)

    # weights in DoubleRow pair layout: [pair j][p, i, m] = wT[(2j+i)*128+p, m]
    qkv_wT = np.ascontiguousarray(qkv_w.T)            # [C, 3C]
    qkv_w8 = qkv_wT.reshape(2, 2, 128, 3 * C).transpose(0, 2, 1, 3)
    proj_wT = np.ascontiguousarray(np.asarray(proj_w, f).T)  # [C, C]

    import ml_dtypes as _mld
    consts = {
        "qkv_w8": np.ascontiguousarray(qkv_w8).astype(E4),
        "proj_wbf": np.ascontiguousarray(
            proj_wT.reshape(CT, 128, C)).astype(_mld.bfloat16),
        "qk_bias_cols": np.ascontiguousarray(
            qkv_b[:2 * C].reshape(2 * CT, 128).T),
        "nw_cols": np.ascontiguousarray(np.asarray(norm_w, f).reshape(CT, 128).T),
        "nb_cols": np.ascontiguousarray(np.asarray(norm_b, f).reshape(CT, 128).T),
        "pb_cols": np.ascontiguousarray(
            (np.asarray(proj_b, f)
             + np.asarray(proj_w, f) @ qkv_b[2 * C:]).reshape(CT, 128).T),
    }
    gmask = np.zeros((128, GPT), f)
    for p in range(128):
        gmask[p, p // GS] = 1.0 / GS
    consts["gmask"] = gmask
    consts["ident"] = np.eye(128, dtype=f)
    consts["bmask"] = np.ascontiguousarray((gmask.T > 0).astype(f))

    xs = np.ascontiguousarray(np.asarray(x, f).reshape(N_CORES, BPC, C, T))
    return xs, consts


_NC_CACHE = None


def kernel(x, norm_w, norm_b, qkv_w, qkv_b, proj_w, proj_b, num_heads=8, **_):
    from concourse.bass_utils import run_bass_kernel_spmd

    assert int(num_heads) == NH
    global _NC_CACHE
    if _NC_CACHE is None:
        _NC_CACHE = build_nc()
    nc = _NC_CACHE

    xs, consts = host_inputs(x, norm_w, norm_b, qkv_w, qkv_b, proj_w, proj_b)
    in_maps = [{"x": xs[i], **consts} for i in range(N_CORES)]
    res = run_bass_kernel_spmd(nc, in_maps, core_ids=list(range(N_CORES)))
    out = np.stack([res.results[i]["out"] for i in range(N_CORES)])
    return out.reshape(B, C, HH, WW)

